# revision 24
# baseline (speedup 1.0000x reference)
"""Trainium2 Bass kernel for nn_AlwGAT (GAT-style message passing), v4.

Math (equivalent to the reference):
  self = x[:, :36]; others = x[:, 36:].reshape(B, 19, 28)
  e_j  = exp(others_j . Wa[36:])        # softmax shift-invariance: self part cancels
  s    = sum_j e_j
  out  = [ self @ A_self + (sum_j (e_j/s) * others_j) @ A_pool ] + c
where
  A_self = We[:36] @ Wo[:64] + (Ws[:36] + Ws[36:]) @ Wo[H:]
  A_pool = We[36:] @ Wo[:64]
  c      = be @ Wo[:64] + bs @ Wo[H:] + bo      (added on host)

Dataflow (feature-major; host pre-transposes x per core to xT[569, 8192] bf16,
feature order = [others(532), self(36), ones(1)]):
  per 512-row group (16 groups/core):
    xt    : 5 feature chunks on partitions (c0-3: 128@0, c4: 57@0), contiguous DMA
    logits: lt[128, 512] = sum_c WL_c^T @ xt_c   (5 accumulating MMs; WL holds the
            19 logit columns replicated into all four 32-row groups)
    exp   : ACT -> eT[128, 512] bf16 (replicas at partitions 32g+j)
    erep  : 4 row-packed concurrent MMs (tile_position=(32g,0)) broadcast e_j to
            feature partitions via 0/1 selector B_g; 5th MM (B4) covers chunk 4:
            others-indicators, ones for self (-> s), ones for the s-column (p56)
    sp    : ONE merged DVE mul [128, 5, 512]: xt * er  (self features get *s,
            the ones-feature becomes s itself)
    final : ops[65, 512] = sum_c FW_c^T @ sp_c  (5 accumulating MMs, N=512;
            col 64 routes the s feature -> ops[64] = s)
    copy  : ACT Copy ops -> bf16 staging; coalesced DMA out per 4 groups
  Device output is s*(true_out - c) stacked with s; host divides and adds c.
"""

import os
import sys

if "/opt/trn_rl_repo" not in sys.path:
    sys.path.insert(0, "/opt/trn_rl_repo")

import numpy as np

SELF = 36
OTH = 28
J = 19
H = 64
H1 = H + 1  # 64 outputs + s column
OBS = SELF + OTH * J  # 568
NOTH = OTH * J  # 532
XR = OBS + 1  # 569 device feature rows (ones row appended)
NCORES = 8
BATCH = 65536
ROWS_PER_CORE = BATCH // NCORES  # 8192
R = 512  # rows per compute group
NG = ROWS_PER_CORE // R  # 16
LG = 4  # compute groups per DMA load group
NL = NG // LG  # 4
RL = R * LG  # 2048 rows per load
C4N = 57  # live partitions in chunk 4 (20 others + 36 self + 1 ones)

_CACHE = {}


def _build_nc():
    import concourse.bass as bass  # noqa: F401
    import concourse.tile as tile
    from concourse import bacc, mybir

    f32 = mybir.dt.float32
    bf16 = mybir.dt.bfloat16

    nc = bacc.Bacc("TRN2", debug=False)
    x_d = nc.dram_tensor("x_in", [XR, ROWS_PER_CORE], bf16, kind="ExternalInput").ap()
    wl_d = nc.dram_tensor("wl_in", [128, 5, 128], f32, kind="ExternalInput").ap()
    b_d = nc.dram_tensor("b_in", [128, 2, 128], f32, kind="ExternalInput").ap()
    fw_d = nc.dram_tensor("fw_in", [128, 5, 128], f32, kind="ExternalInput").ap()
    out_d = nc.dram_tensor("out", [H1, ROWS_PER_CORE], bf16, kind="ExternalOutput").ap()

    Exp = mybir.ActivationFunctionType.Exp
    Copy = mybir.ActivationFunctionType.Copy

    with tile.TileContext(nc) as tc:
        with (
            tc.tile_pool(name="consts", bufs=1) as consts,
            tc.tile_pool(name="xt", bufs=4) as xt_pool,
            tc.tile_pool(name="eT", bufs=2) as eT_pool,
            tc.tile_pool(name="spa", bufs=3) as sp_pool,
            tc.tile_pool(name="spb", bufs=3) as spb_pool,
            tc.tile_pool(name="osb", bufs=2) as osb_pool,
            tc.tile_pool(name="psLT", bufs=2, space="PSUM") as lt_pool,
            tc.tile_pool(name="psERA", bufs=1, space="PSUM") as era_pool,
            tc.tile_pool(name="psERB", bufs=1, space="PSUM") as erb_pool,
            tc.tile_pool(name="psOUT", bufs=1, space="PSUM") as op_pool,
        ):
            # constants: stage f32, convert once to bf16
            wl_st = consts.tile([128, 5, 128], f32)
            nc.sync.dma_start(out=wl_st, in_=wl_d)
            wl_sb = consts.tile([128, 5, 128], bf16)
            nc.scalar.copy(out=wl_sb, in_=wl_st)
            b_st = consts.tile([128, 2, 128], f32)
            nc.sync.dma_start(out=b_st, in_=b_d)
            b_sb = consts.tile([128, 2, 128], bf16)
            nc.scalar.copy(out=b_sb, in_=b_st)
            fw_st = consts.tile([128, 5, 128], f32)
            nc.sync.dma_start(out=fw_st, in_=fw_d)
            fw_sb = consts.tile([128, 5, 128], bf16)
            nc.scalar.copy(out=fw_sb, in_=fw_st)

            st = {}

            def do_load(ld):
                r0 = ld * RL
                xb = xt_pool.tile([128, 5, RL], bf16, tag="xt")
                # first load split per compute group so group 0 can start ASAP
                pieces = range(LG) if ld == 0 else [None]
                for pc in pieces:
                    s = slice(0, RL) if pc is None else slice(R * pc, R * (pc + 1))
                    ra = slice(r0 + s.start, r0 + s.stop)
                    for c in range(4):
                        q = nc.sync if c % 2 == 0 else nc.scalar
                        q.dma_start(out=xb[:, c, s], in_=x_d[128 * c : 128 * (c + 1), ra])
                    nc.sync.dma_start(out=xb[0:C4N, 4, s], in_=x_d[512:XR, ra])
                # partitions C4N..127 of chunk 4 stay uninitialized; the merged
                # DVE mul multiplies them by er=0 and nothing reads the result
                st[("xb", ld)] = xb
                ob = osb_pool.tile([H1, LG, R], bf16, tag="osb")
                st[("ob", ld)] = ob

            def s_logits(t):
                xb = st[("xb", t // LG)]
                sl = slice(R * (t % LG), R * (t % LG) + R)
                lt = lt_pool.tile([128, R], f32, tag="lt")
                for c in range(4):
                    nc.tensor.matmul(
                        lt, wl_sb[:, c, :], xb[:, c, sl], start=(c == 0), stop=False
                    )
                nc.tensor.matmul(
                    lt, wl_sb[0:C4N, 4, :], xb[0:C4N, 4, sl], start=False, stop=True
                )
                st[("lt", t)] = lt

            def s_exp(t):
                lt = st.pop(("lt", t))
                eT = eT_pool.tile([128, R], bf16, tag="eT")
                nc.scalar.activation(out=eT, in_=lt, func=Exp)
                st[("eT", t)] = eT

            def s_final(t):
                # final MMs for group t, emitted two iterations later so the PE
                # never head-of-line blocks on the DVE sp of the same group
                spa = st.pop(("spa", t))
                spb = st.pop(("spb", t))
                ops = op_pool.tile([128, R], f32, tag="ops")
                for c in range(4):
                    nc.tensor.matmul(
                        ops, fw_sb[:, c, :], spa[:, c, :], start=(c == 0), stop=False
                    )
                nc.tensor.matmul(
                    ops, fw_sb[0:C4N, 4, :], spb[0:C4N, :], start=False, stop=True
                )
                st[("ops", t)] = ops

            def s_erep_a(t):
                eT = st[("eT", t)]
                era = era_pool.tile([128, 4, R], f32, tag="era")
                for g in range(4):
                    nc.tensor.matmul(
                        era[:, g, :],
                        b_sb[32 * g : 32 * g + J, 0, :],
                        eT[32 * g : 32 * g + J, :],
                        start=True,
                        stop=True,
                        tile_position=(32 * g, 0),
                    )
                st[("era", t)] = era

            def s_erep_b(t):
                eT = st.pop(("eT", t))
                erb = erb_pool.tile([128, R], f32, tag="erb")
                nc.tensor.matmul(erb, b_sb[0:J, 1, :], eT[0:J, :], start=True, stop=True)
                st[("erb", t)] = erb

            def s_sp_a(t):
                xb = st[("xb", t // LG)]
                sl = slice(R * (t % LG), R * (t % LG) + R)
                era = st.pop(("era", t))
                spa = sp_pool.tile([128, 4, R], bf16, tag="spa")
                nc.vector.tensor_mul(spa, xb[:, 0:4, sl], era)
                st[("spa", t)] = spa

            def s_sp_b(t):
                xb = st[("xb", t // LG)]
                sl = slice(R * (t % LG), R * (t % LG) + R)
                erb = st.pop(("erb", t))
                spb = spb_pool.tile([128, R], bf16, tag="spb")
                nc.vector.tensor_mul(spb, xb[:, 4, sl], erb)
                st[("spb", t)] = spb

            def s_copy(t):
                ops = st.pop(("ops", t))
                ob = st[("ob", t // LG)]
                nc.scalar.activation(out=ob[:, t % LG, :], in_=ops[0:H1, :], func=Copy)

            def s_store(t):
                if t % LG == LG - 1:
                    ld = t // LG
                    r0 = ld * RL
                    nc.gpsimd.dma_start(
                        out=out_d[:, r0 : r0 + RL], in_=st.pop(("ob", ld))
                    )
                    st.pop(("xb", ld), None)

            for ld in range(NL):
                do_load(ld)
            # dummy matmuls to warm the PE HAM clock gate while the first
            # x DMAs are in flight (cold->warm is ~3.4us of sustained busy)
            warm = lt_pool.tile([128, R], f32, tag="lt")
            for _ in range(48):
                nc.tensor.matmul(
                    warm[:, 0:H], wl_sb[:, 0, :], b_sb[:, 0, 0:H], start=True, stop=True
                )
            stages = [
                (s_logits, 0),
                (s_exp, 0),
                (s_final, 2),
                (s_erep_a, 0),
                (s_erep_b, 0),
                (s_sp_a, 0),
                (s_sp_b, 0),
                (s_copy, 2),
                (s_store, 2),
            ]
            for r in range(NG + 2):
                for fn, off in stages:
                    tt = r - off
                    if 0 <= tt < NG:
                        fn(tt)

    nc.compile()
    return nc


def _fold_weights(Wa, ba, We, be, Ws, bs, Wo, bo):
    Wa = np.asarray(Wa, np.float64)
    We = np.asarray(We, np.float64)
    Ws = np.asarray(Ws, np.float64)
    Wo = np.asarray(Wo, np.float64)
    wa2 = Wa[SELF:, 0]  # [28]
    A_self = We[:SELF] @ Wo[:H] + (Ws[:SELF] + Ws[SELF:]) @ Wo[H:]  # [36, 64]
    A_pool = We[SELF:] @ Wo[:H]  # [28, 64]
    c = (
        np.asarray(be, np.float64) @ Wo[:H]
        + np.asarray(bs, np.float64) @ Wo[H:]
        + np.asarray(bo, np.float64)
    )  # [64]

    # feature-major order: f_or = 28*j + k for others, then self, then ones
    WLp = np.zeros((128, 5, 128), np.float32)
    Bp = np.zeros((128, 2, 128), np.float32)
    FWp = np.zeros((128, 5, 128), np.float32)
    for ch in range(4):
        for p in range(128):
            f_or = 128 * ch + p
            j, k = divmod(f_or, OTH)
            for g in range(4):
                WLp[p, ch, 32 * g + j] = wa2[k]
            FWp[p, ch, 0:H] = A_pool[k]
    for g in range(4):
        for p in range(128):
            j = (128 * g + p) // OTH
            Bp[32 * g + j, 0, p] = 1.0
    # chunk 4: partitions 0..19 = others f_or 512..531, 20..55 = self, 56 = ones
    for i in range(20):
        f_or = 512 + i
        j, k = divmod(f_or, OTH)
        for g in range(4):
            WLp[i, 4, 32 * g + j] = wa2[k]
        Bp[j, 1, i] = 1.0
        FWp[i, 4, 0:H] = A_pool[k]
    for t in range(SELF):
        p = 20 + t
        Bp[0:J, 1, p] = 1.0  # ones -> er4 = s on self partitions
        FWp[p, 4, 0:H] = A_self[t]
    Bp[0:J, 1, 56] = 1.0  # ones -> er4[56] = s; x ones-row makes sp[56] = s
    FWp[56, 4, H] = 1.0  # route s into ops[64]
    return WLp, Bp, FWp, c.astype(np.float32)


def kernel(x, Wa, ba, We, be, Ws, bs, Wo, bo):
    import ml_dtypes

    from concourse import bass_utils

    x = np.asarray(x, np.float32)
    assert x.shape == (BATCH, OBS), x.shape
    # host-side: bf16 cast + per-core feature-major transpose
    # feature order: others (x[:, 36:]) then self (x[:, :36]) then ones
    xb = x.astype(ml_dtypes.bfloat16)
    xT = np.empty((XR, BATCH), dtype=ml_dtypes.bfloat16)
    xT[0:NOTH] = xb[:, SELF:].T
    xT[NOTH:OBS] = xb[:, 0:SELF].T
    xT[OBS] = 1.0

    WLp, Bp, FWp, c = _fold_weights(Wa, ba, We, be, Ws, bs, Wo, bo)

    if "nc" not in _CACHE:
        _CACHE["nc"] = _build_nc()
    nc = _CACHE["nc"]

    in_maps = []
    for i in range(NCORES):
        in_maps.append(
            {
                "x_in": np.ascontiguousarray(
                    xT[:, i * ROWS_PER_CORE : (i + 1) * ROWS_PER_CORE]
                ),
                "wl_in": WLp,
                "b_in": Bp,
                "fw_in": FWp,
            }
        )

    res = bass_utils.run_bass_kernel_spmd(
        nc,
        in_maps,
        core_ids=list(range(NCORES)),
        trace=_CACHE.get("trace", False),
        **_CACHE.get("run_kwargs", {}),
    )
    _CACHE["last_results"] = res

    # out_d is [65, 8192] per core: rows 0..63 = s*(out-c), row 64 = s
    out = np.concatenate(
        [np.asarray(res.results[i]["out"]).astype(np.float32).T for i in range(NCORES)],
        0,
    )
    out = out[:, 0:H] / out[:, H : H + 1]
    out = out + c[None, :]
    return out.astype(np.float32)


# revision 29
# speedup vs baseline: 1.0613x; 1.0613x over previous
"""Trainium2 Bass kernel for nn_AlwGAT (GAT-style message passing), v4.

Math (equivalent to the reference):
  self = x[:, :36]; others = x[:, 36:].reshape(B, 19, 28)
  e_j  = exp(others_j . Wa[36:])        # softmax shift-invariance: self part cancels
  s    = sum_j e_j
  out  = [ self @ A_self + (sum_j (e_j/s) * others_j) @ A_pool ] + c
where
  A_self = We[:36] @ Wo[:64] + (Ws[:36] + Ws[36:]) @ Wo[H:]
  A_pool = We[36:] @ Wo[:64]
  c      = be @ Wo[:64] + bs @ Wo[H:] + bo      (added on host)

Dataflow (feature-major; host pre-transposes x per core to xT[569, 8192] bf16,
feature order = [others(532), self(36), ones(1)]):
  per 512-row group (16 groups/core):
    xt    : 5 feature chunks on partitions (c0-3: 128@0, c4: 57@0), contiguous DMA
    logits: lt[128, 512] = sum_c WL_c^T @ xt_c   (5 accumulating MMs; WL holds the
            19 logit columns replicated into all four 32-row groups)
    exp   : ACT -> eT[128, 512] bf16 (replicas at partitions 32g+j)
    erep  : 4 row-packed concurrent MMs (tile_position=(32g,0)) broadcast e_j to
            feature partitions via 0/1 selector B_g; 5th MM (B4) covers chunk 4:
            others-indicators, ones for self (-> s), ones for the s-column (p56)
    sp    : ONE merged DVE mul [128, 5, 512]: xt * er  (self features get *s,
            the ones-feature becomes s itself)
    final : ops[65, 512] = sum_c FW_c^T @ sp_c  (5 accumulating MMs, N=512;
            col 64 routes the s feature -> ops[64] = s)
    copy  : ACT Copy ops -> bf16 staging; coalesced DMA out per 4 groups
  Device output is s*(true_out - c) stacked with s; host divides and adds c.
"""

import os
import sys

if "/opt/trn_rl_repo" not in sys.path:
    sys.path.insert(0, "/opt/trn_rl_repo")

import numpy as np

SELF = 36
OTH = 28
J = 19
H = 64
H1 = H + 1  # 64 outputs + s column
OBS = SELF + OTH * J  # 568
NOTH = OTH * J  # 532
XR = OBS + 1  # 569 device feature rows (ones row appended)
NCORES = 8
BATCH = 65536
ROWS_PER_CORE = BATCH // NCORES  # 8192
R = 512  # rows per compute group
NG = ROWS_PER_CORE // R  # 16
LG = 4  # compute groups per DMA load group
NL = NG // LG  # 4
RL = R * LG  # 2048 rows per load
C4N = 57  # live partitions in chunk 4 (20 others + 36 self + 1 ones)

_CACHE = {}


def _build_nc():
    import concourse.bass as bass  # noqa: F401
    import concourse.tile as tile
    from concourse import bacc, mybir

    f32 = mybir.dt.float32
    bf16 = mybir.dt.bfloat16

    nc = bacc.Bacc("TRN2", debug=False)
    x_d = nc.dram_tensor("x_in", [XR, ROWS_PER_CORE], bf16, kind="ExternalInput").ap()
    wl_d = nc.dram_tensor("wl_in", [128, 5, 128], f32, kind="ExternalInput").ap()
    b_d = nc.dram_tensor("b_in", [128, 2, 128], f32, kind="ExternalInput").ap()
    fw_d = nc.dram_tensor("fw_in", [128, 5, 128], f32, kind="ExternalInput").ap()
    out_d = nc.dram_tensor("out", [H1, ROWS_PER_CORE], bf16, kind="ExternalOutput").ap()

    Exp = mybir.ActivationFunctionType.Exp
    Copy = mybir.ActivationFunctionType.Copy

    with tile.TileContext(nc) as tc:
        with (
            tc.tile_pool(name="consts", bufs=1) as consts,
            tc.tile_pool(name="xt", bufs=4) as xt_pool,
            tc.tile_pool(name="eT", bufs=2) as eT_pool,
            tc.tile_pool(name="sp", bufs=3) as sp_pool,
            tc.tile_pool(name="osb", bufs=2) as osb_pool,
            tc.tile_pool(name="psLT", bufs=2, space="PSUM") as lt_pool,
            tc.tile_pool(name="psER", bufs=1, space="PSUM") as er_pool,
            tc.tile_pool(name="psOUT", bufs=1, space="PSUM") as op_pool,
        ):
            # constants: stage f32, convert once to bf16
            wl_st = consts.tile([128, 5, 128], f32)
            nc.sync.dma_start(out=wl_st, in_=wl_d)
            wl_sb = consts.tile([128, 5, 128], bf16)
            nc.scalar.copy(out=wl_sb, in_=wl_st)
            b_st = consts.tile([128, 2, 128], f32)
            nc.sync.dma_start(out=b_st, in_=b_d)
            b_sb = consts.tile([128, 2, 128], bf16)
            nc.scalar.copy(out=b_sb, in_=b_st)
            fw_st = consts.tile([128, 5, 128], f32)
            nc.sync.dma_start(out=fw_st, in_=fw_d)
            fw_sb = consts.tile([128, 5, 128], bf16)
            nc.scalar.copy(out=fw_sb, in_=fw_st)

            st = {}

            qs = [nc.sync, nc.scalar, nc.gpsimd]
            qi = [0]

            def nextq():
                q = qs[qi[0] % 3]
                qi[0] += 1
                return q

            def do_load(ld):
                r0 = ld * RL
                xb = xt_pool.tile([128, 5, RL], bf16, tag="xt")
                # first load split per compute group so group 0 can start ASAP;
                # DMA issues round-robin over 4 engine queues (issue overhead
                # ~600ns each would otherwise serialize the start)
                pieces = range(LG) if ld == 0 else [None]
                for pc in pieces:
                    s = slice(0, RL) if pc is None else slice(R * pc, R * (pc + 1))
                    ra = slice(r0 + s.start, r0 + s.stop)
                    for c in range(4):
                        nextq().dma_start(
                            out=xb[:, c, s], in_=x_d[128 * c : 128 * (c + 1), ra]
                        )
                    nextq().dma_start(out=xb[0:C4N, 4, s], in_=x_d[512:XR, ra])
                # partitions C4N..127 of chunk 4 stay uninitialized; the merged
                # DVE mul multiplies them by er=0 and nothing reads the result
                st[("xb", ld)] = xb
                ob = osb_pool.tile([H1, LG, R], bf16, tag="osb")
                st[("ob", ld)] = ob

            def s_logits(t):
                xb = st[("xb", t // LG)]
                sl = slice(R * (t % LG), R * (t % LG) + R)
                lt = lt_pool.tile([128, R], f32, tag="lt")
                for c in range(4):
                    nc.tensor.matmul(
                        lt, wl_sb[:, c, :], xb[:, c, sl], start=(c == 0), stop=False
                    )
                nc.tensor.matmul(
                    lt, wl_sb[0:C4N, 4, :], xb[0:C4N, 4, sl], start=False, stop=True
                )
                st[("lt", t)] = lt

            def s_exp(t):
                lt = st.pop(("lt", t))
                eT = eT_pool.tile([128, R], bf16, tag="eT")
                nc.scalar.activation(out=eT, in_=lt, func=Exp)
                st[("eT", t)] = eT

            def s_final(t):
                # final MMs for group t, emitted two iterations later so the PE
                # never head-of-line blocks on the DVE sp of the same group
                sp = st.pop(("sp", t))
                ops = op_pool.tile([128, R], f32, tag="ops")
                for c in range(4):
                    nc.tensor.matmul(
                        ops, fw_sb[:, c, :], sp[:, c, :], start=(c == 0), stop=False
                    )
                nc.tensor.matmul(
                    ops, fw_sb[0:C4N, 4, :], sp[0:C4N, 4, :], start=False, stop=True
                )
                st[("ops", t)] = ops

            def s_erep(t):
                eT = st.pop(("eT", t))
                er = er_pool.tile([128, 5, R], f32, tag="er")
                for g in range(4):
                    nc.tensor.matmul(
                        er[:, g, :],
                        b_sb[32 * g : 32 * g + J, 0, :],
                        eT[32 * g : 32 * g + J, :],
                        start=True,
                        stop=True,
                        tile_position=(32 * g, 0),
                    )
                nc.tensor.matmul(
                    er[:, 4, :], b_sb[0:J, 1, :], eT[0:J, :], start=True, stop=True
                )
                st[("er", t)] = er

            def s_sp(t):
                xb = st[("xb", t // LG)]
                sl = slice(R * (t % LG), R * (t % LG) + R)
                er = st.pop(("er", t))
                sp = sp_pool.tile([128, 5, R], bf16, tag="sp")
                nc.vector.tensor_mul(sp, xb[:, 0:5, sl], er)
                st[("sp", t)] = sp

            def s_copy(t):
                ops = st.pop(("ops", t))
                ob = st[("ob", t // LG)]
                nc.scalar.activation(out=ob[:, t % LG, :], in_=ops[0:H1, :], func=Copy)

            def s_store(t):
                if t % LG == LG - 1:
                    ld = t // LG
                    r0 = ld * RL
                    nc.gpsimd.dma_start(
                        out=out_d[:, r0 : r0 + RL], in_=st.pop(("ob", ld))
                    )
                    st.pop(("xb", ld), None)

            for ld in range(NL):
                do_load(ld)
            # dummy matmuls to warm the PE HAM clock gate while the first
            # x DMAs are in flight (cold->warm is ~3.4us of sustained busy)
            warm = lt_pool.tile([128, R], f32, tag="lt")
            for _ in range(48):
                nc.tensor.matmul(
                    warm[:, 0:H], wl_sb[:, 0, :], b_sb[:, 0, 0:H], start=True, stop=True
                )
            stages = [
                (s_logits, 0),
                (s_exp, 0),
                (s_final, 2),
                (s_erep, 0),
                (s_sp, 0),
                (s_copy, 2),
                (s_store, 2),
            ]
            for r in range(NG + 2):
                for fn, off in stages:
                    tt = r - off
                    if 0 <= tt < NG:
                        fn(tt)

    nc.compile()
    return nc


def _fold_weights(Wa, ba, We, be, Ws, bs, Wo, bo):
    Wa = np.asarray(Wa, np.float64)
    We = np.asarray(We, np.float64)
    Ws = np.asarray(Ws, np.float64)
    Wo = np.asarray(Wo, np.float64)
    wa2 = Wa[SELF:, 0]  # [28]
    A_self = We[:SELF] @ Wo[:H] + (Ws[:SELF] + Ws[SELF:]) @ Wo[H:]  # [36, 64]
    A_pool = We[SELF:] @ Wo[:H]  # [28, 64]
    c = (
        np.asarray(be, np.float64) @ Wo[:H]
        + np.asarray(bs, np.float64) @ Wo[H:]
        + np.asarray(bo, np.float64)
    )  # [64]

    # feature-major order: f_or = 28*j + k for others, then self, then ones
    WLp = np.zeros((128, 5, 128), np.float32)
    Bp = np.zeros((128, 2, 128), np.float32)
    FWp = np.zeros((128, 5, 128), np.float32)
    for ch in range(4):
        for p in range(128):
            f_or = 128 * ch + p
            j, k = divmod(f_or, OTH)
            for g in range(4):
                WLp[p, ch, 32 * g + j] = wa2[k]
            FWp[p, ch, 0:H] = A_pool[k]
    for g in range(4):
        for p in range(128):
            j = (128 * g + p) // OTH
            Bp[32 * g + j, 0, p] = 1.0
    # chunk 4: partitions 0..19 = others f_or 512..531, 20..55 = self, 56 = ones
    for i in range(20):
        f_or = 512 + i
        j, k = divmod(f_or, OTH)
        for g in range(4):
            WLp[i, 4, 32 * g + j] = wa2[k]
        Bp[j, 1, i] = 1.0
        FWp[i, 4, 0:H] = A_pool[k]
    for t in range(SELF):
        p = 20 + t
        Bp[0:J, 1, p] = 1.0  # ones -> er4 = s on self partitions
        FWp[p, 4, 0:H] = A_self[t]
    Bp[0:J, 1, 56] = 1.0  # ones -> er4[56] = s; x ones-row makes sp[56] = s
    FWp[56, 4, H] = 1.0  # route s into ops[64]
    return WLp, Bp, FWp, c.astype(np.float32)


def kernel(x, Wa, ba, We, be, Ws, bs, Wo, bo):
    import ml_dtypes

    from concourse import bass_utils

    x = np.asarray(x, np.float32)
    assert x.shape == (BATCH, OBS), x.shape
    # host-side: bf16 cast + per-core feature-major transpose
    # feature order: others (x[:, 36:]) then self (x[:, :36]) then ones
    xb = x.astype(ml_dtypes.bfloat16)
    xT = np.empty((XR, BATCH), dtype=ml_dtypes.bfloat16)
    xT[0:NOTH] = xb[:, SELF:].T
    xT[NOTH:OBS] = xb[:, 0:SELF].T
    xT[OBS] = 1.0

    WLp, Bp, FWp, c = _fold_weights(Wa, ba, We, be, Ws, bs, Wo, bo)

    if "nc" not in _CACHE:
        _CACHE["nc"] = _build_nc()
    nc = _CACHE["nc"]

    in_maps = []
    for i in range(NCORES):
        in_maps.append(
            {
                "x_in": np.ascontiguousarray(
                    xT[:, i * ROWS_PER_CORE : (i + 1) * ROWS_PER_CORE]
                ),
                "wl_in": WLp,
                "b_in": Bp,
                "fw_in": FWp,
            }
        )

    res = bass_utils.run_bass_kernel_spmd(
        nc,
        in_maps,
        core_ids=list(range(NCORES)),
        trace=_CACHE.get("trace", False),
        **_CACHE.get("run_kwargs", {}),
    )
    _CACHE["last_results"] = res

    # out_d is [65, 8192] per core: rows 0..63 = s*(out-c), row 64 = s
    out = np.concatenate(
        [np.asarray(res.results[i]["out"]).astype(np.float32).T for i in range(NCORES)],
        0,
    )
    out = out[:, 0:H] / out[:, H : H + 1]
    out = out + c[None, :]
    return out.astype(np.float32)


# revision 31
# speedup vs baseline: 1.0818x; 1.0192x over previous
"""Trainium2 Bass kernel for nn_AlwGAT (GAT-style message passing), v4.

Math (equivalent to the reference):
  self = x[:, :36]; others = x[:, 36:].reshape(B, 19, 28)
  e_j  = exp(others_j . Wa[36:])        # softmax shift-invariance: self part cancels
  s    = sum_j e_j
  out  = [ self @ A_self + (sum_j (e_j/s) * others_j) @ A_pool ] + c
where
  A_self = We[:36] @ Wo[:64] + (Ws[:36] + Ws[36:]) @ Wo[H:]
  A_pool = We[36:] @ Wo[:64]
  c      = be @ Wo[:64] + bs @ Wo[H:] + bo      (added on host)

Dataflow (feature-major; host pre-transposes x per core to xT[569, 8192] bf16,
feature order = [others(532), self(36), ones(1)]):
  per 512-row group (16 groups/core):
    xt    : 5 feature chunks on partitions (c0-3: 128@0, c4: 57@0), contiguous DMA
    logits: lt[128, 512] = sum_c WL_c^T @ xt_c   (5 accumulating MMs; WL holds the
            19 logit columns replicated into all four 32-row groups)
    exp   : ACT -> eT[128, 512] bf16 (replicas at partitions 32g+j)
    erep  : 4 row-packed concurrent MMs (tile_position=(32g,0)) broadcast e_j to
            feature partitions via 0/1 selector B_g; 5th MM (B4) covers chunk 4:
            others-indicators, ones for self (-> s), ones for the s-column (p56)
    sp    : ONE merged DVE mul [128, 5, 512]: xt * er  (self features get *s,
            the ones-feature becomes s itself)
    final : ops[65, 512] = sum_c FW_c^T @ sp_c  (5 accumulating MMs, N=512;
            col 64 routes the s feature -> ops[64] = s)
    copy  : ACT Copy ops -> bf16 staging; coalesced DMA out per 4 groups
  Device output is s*(true_out - c) stacked with s; host divides and adds c.
"""

import os
import sys

if "/opt/trn_rl_repo" not in sys.path:
    sys.path.insert(0, "/opt/trn_rl_repo")

import numpy as np

SELF = 36
OTH = 28
J = 19
H = 64
H1 = H + 1  # 64 outputs + s column
OBS = SELF + OTH * J  # 568
NOTH = OTH * J  # 532
XR = OBS + 1  # 569 device feature rows (ones row appended)
NCORES = 8
BATCH = 65536
ROWS_PER_CORE = BATCH // NCORES  # 8192
R = 512  # rows per compute group
NG = ROWS_PER_CORE // R  # 16
LG = 4  # compute groups per DMA load group
NL = NG // LG  # 4
RL = R * LG  # 2048 rows per load
C4N = 57  # live partitions in chunk 4 (20 others + 36 self + 1 ones)

_CACHE = {}


def _build_nc():
    import concourse.bass as bass  # noqa: F401
    import concourse.tile as tile
    from concourse import bacc, mybir

    f32 = mybir.dt.float32
    bf16 = mybir.dt.bfloat16

    nc = bacc.Bacc("TRN2", debug=False)
    x_d = nc.dram_tensor("x_in", [XR, ROWS_PER_CORE], bf16, kind="ExternalInput").ap()
    wl_d = nc.dram_tensor("wl_in", [128, 5, 128], f32, kind="ExternalInput").ap()
    b_d = nc.dram_tensor("b_in", [128, 2, 128], f32, kind="ExternalInput").ap()
    fw_d = nc.dram_tensor("fw_in", [128, 5, 128], f32, kind="ExternalInput").ap()
    out_d = nc.dram_tensor("out", [H1, ROWS_PER_CORE], bf16, kind="ExternalOutput").ap()

    Exp = mybir.ActivationFunctionType.Exp
    Copy = mybir.ActivationFunctionType.Copy

    with tile.TileContext(nc) as tc:
        with (
            tc.tile_pool(name="consts", bufs=1) as consts,
            tc.tile_pool(name="xt", bufs=4) as xt_pool,
            tc.tile_pool(name="eT", bufs=2) as eT_pool,
            tc.tile_pool(name="sp", bufs=3) as sp_pool,
            tc.tile_pool(name="osb", bufs=2) as osb_pool,
            tc.tile_pool(name="psLT", bufs=2, space="PSUM") as lt_pool,
            tc.tile_pool(name="psER", bufs=1, space="PSUM") as er_pool,
            tc.tile_pool(name="psOUT", bufs=1, space="PSUM") as op_pool,
        ):
            # constants: stage f32, convert once to bf16
            wl_st = consts.tile([128, 5, 128], f32)
            nc.sync.dma_start(out=wl_st, in_=wl_d)
            wl_sb = consts.tile([128, 5, 128], bf16)
            nc.scalar.copy(out=wl_sb, in_=wl_st)
            b_st = consts.tile([128, 2, 128], f32)
            nc.sync.dma_start(out=b_st, in_=b_d)
            b_sb = consts.tile([128, 2, 128], bf16)
            nc.scalar.copy(out=b_sb, in_=b_st)
            fw_st = consts.tile([128, 5, 128], f32)
            nc.sync.dma_start(out=fw_st, in_=fw_d)
            fw_sb = consts.tile([128, 5, 128], bf16)
            nc.scalar.copy(out=fw_sb, in_=fw_st)

            st = {}

            qs = [nc.sync, nc.scalar]
            qi = [0]

            def nextq():
                q = qs[qi[0] % 2]
                qi[0] += 1
                return q

            def do_load(ld):
                r0 = ld * RL
                xb = xt_pool.tile([128, 5, RL], bf16, tag="xt")
                # first load split per compute group so group 0 can start ASAP;
                # DMA issues round-robin over 4 engine queues (issue overhead
                # ~600ns each would otherwise serialize the start)
                pieces = range(LG) if ld == 0 else [None]
                for pc in pieces:
                    s = slice(0, RL) if pc is None else slice(R * pc, R * (pc + 1))
                    ra = slice(r0 + s.start, r0 + s.stop)
                    for c in range(4):
                        nextq().dma_start(
                            out=xb[:, c, s], in_=x_d[128 * c : 128 * (c + 1), ra]
                        )
                    nextq().dma_start(out=xb[0:C4N, 4, s], in_=x_d[512:XR, ra])
                # partitions C4N..127 of chunk 4 stay uninitialized; the merged
                # DVE mul multiplies them by er=0 and nothing reads the result
                st[("xb", ld)] = xb
                ob = osb_pool.tile([H1, LG, R], bf16, tag="osb")
                st[("ob", ld)] = ob

            def s_logits(t):
                xb = st[("xb", t // LG)]
                sl = slice(R * (t % LG), R * (t % LG) + R)
                lt = lt_pool.tile([128, R], f32, tag="lt")
                for c in range(4):
                    nc.tensor.matmul(
                        lt, wl_sb[:, c, :], xb[:, c, sl], start=(c == 0), stop=False
                    )
                nc.tensor.matmul(
                    lt, wl_sb[0:C4N, 4, :], xb[0:C4N, 4, sl], start=False, stop=True
                )
                st[("lt", t)] = lt

            def s_exp(t):
                lt = st.pop(("lt", t))
                eT = eT_pool.tile([128, R], bf16, tag="eT")
                nc.scalar.activation(out=eT, in_=lt, func=Exp)
                st[("eT", t)] = eT

            def s_final(t):
                # final MMs for group t, emitted two iterations later so the PE
                # never head-of-line blocks on the DVE sp of the same group
                sp = st.pop(("sp", t))
                ops = op_pool.tile([128, R], f32, tag="ops")
                for c in range(4):
                    nc.tensor.matmul(
                        ops, fw_sb[:, c, :], sp[:, c, :], start=(c == 0), stop=False
                    )
                nc.tensor.matmul(
                    ops, fw_sb[0:C4N, 4, :], sp[0:C4N, 4, :], start=False, stop=True
                )
                st[("ops", t)] = ops

            def s_erep(t):
                eT = st.pop(("eT", t))
                er = er_pool.tile([128, 5, R], f32, tag="er")
                for g in range(4):
                    nc.tensor.matmul(
                        er[:, g, :],
                        b_sb[32 * g : 32 * g + J, 0, :],
                        eT[32 * g : 32 * g + J, :],
                        start=True,
                        stop=True,
                        tile_position=(32 * g, 0),
                    )
                nc.tensor.matmul(
                    er[:, 4, :], b_sb[0:J, 1, :], eT[0:J, :], start=True, stop=True
                )
                st[("er", t)] = er

            def s_sp(t):
                xb = st[("xb", t // LG)]
                sl = slice(R * (t % LG), R * (t % LG) + R)
                er = st.pop(("er", t))
                sp = sp_pool.tile([128, 5, R], bf16, tag="sp")
                nc.vector.tensor_mul(sp, xb[:, 0:5, sl], er)
                st[("sp", t)] = sp

            def s_copy(t):
                ops = st.pop(("ops", t))
                ob = st[("ob", t // LG)]
                nc.scalar.activation(out=ob[:, t % LG, :], in_=ops[0:H1, :], func=Copy)

            def s_store(t):
                if t % LG == LG - 1:
                    ld = t // LG
                    r0 = ld * RL
                    nc.sync.dma_start(
                        out=out_d[:, r0 : r0 + RL], in_=st.pop(("ob", ld))
                    )
                    st.pop(("xb", ld), None)

            for ld in range(NL):
                do_load(ld)
            # dummy matmuls to warm the PE HAM clock gate while the first
            # x DMAs are in flight (cold->warm is ~3.4us of sustained busy)
            warm = lt_pool.tile([128, R], f32, tag="lt")
            for _ in range(48):
                nc.tensor.matmul(
                    warm[:, 0:H], wl_sb[:, 0, :], b_sb[:, 0, 0:H], start=True, stop=True
                )
            stages = [
                (s_logits, 0),
                (s_exp, 0),
                (s_final, 2),
                (s_erep, 0),
                (s_sp, 0),
                (s_copy, 2),
                (s_store, 2),
            ]
            for r in range(NG + 2):
                for fn, off in stages:
                    tt = r - off
                    if 0 <= tt < NG:
                        fn(tt)

    nc.compile()
    return nc


def _fold_weights(Wa, ba, We, be, Ws, bs, Wo, bo):
    Wa = np.asarray(Wa, np.float64)
    We = np.asarray(We, np.float64)
    Ws = np.asarray(Ws, np.float64)
    Wo = np.asarray(Wo, np.float64)
    wa2 = Wa[SELF:, 0]  # [28]
    A_self = We[:SELF] @ Wo[:H] + (Ws[:SELF] + Ws[SELF:]) @ Wo[H:]  # [36, 64]
    A_pool = We[SELF:] @ Wo[:H]  # [28, 64]
    c = (
        np.asarray(be, np.float64) @ Wo[:H]
        + np.asarray(bs, np.float64) @ Wo[H:]
        + np.asarray(bo, np.float64)
    )  # [64]

    # feature-major order: f_or = 28*j + k for others, then self, then ones
    WLp = np.zeros((128, 5, 128), np.float32)
    Bp = np.zeros((128, 2, 128), np.float32)
    FWp = np.zeros((128, 5, 128), np.float32)
    for ch in range(4):
        for p in range(128):
            f_or = 128 * ch + p
            j, k = divmod(f_or, OTH)
            for g in range(4):
                WLp[p, ch, 32 * g + j] = wa2[k]
            FWp[p, ch, 0:H] = A_pool[k]
    for g in range(4):
        for p in range(128):
            j = (128 * g + p) // OTH
            Bp[32 * g + j, 0, p] = 1.0
    # chunk 4: partitions 0..19 = others f_or 512..531, 20..55 = self, 56 = ones
    for i in range(20):
        f_or = 512 + i
        j, k = divmod(f_or, OTH)
        for g in range(4):
            WLp[i, 4, 32 * g + j] = wa2[k]
        Bp[j, 1, i] = 1.0
        FWp[i, 4, 0:H] = A_pool[k]
    for t in range(SELF):
        p = 20 + t
        Bp[0:J, 1, p] = 1.0  # ones -> er4 = s on self partitions
        FWp[p, 4, 0:H] = A_self[t]
    Bp[0:J, 1, 56] = 1.0  # ones -> er4[56] = s; x ones-row makes sp[56] = s
    FWp[56, 4, H] = 1.0  # route s into ops[64]
    return WLp, Bp, FWp, c.astype(np.float32)


def kernel(x, Wa, ba, We, be, Ws, bs, Wo, bo):
    import ml_dtypes

    from concourse import bass_utils

    x = np.asarray(x, np.float32)
    assert x.shape == (BATCH, OBS), x.shape
    # host-side: bf16 cast + per-core feature-major transpose
    # feature order: others (x[:, 36:]) then self (x[:, :36]) then ones
    xb = x.astype(ml_dtypes.bfloat16)
    xT = np.empty((XR, BATCH), dtype=ml_dtypes.bfloat16)
    xT[0:NOTH] = xb[:, SELF:].T
    xT[NOTH:OBS] = xb[:, 0:SELF].T
    xT[OBS] = 1.0

    WLp, Bp, FWp, c = _fold_weights(Wa, ba, We, be, Ws, bs, Wo, bo)

    if "nc" not in _CACHE:
        _CACHE["nc"] = _build_nc()
    nc = _CACHE["nc"]

    in_maps = []
    for i in range(NCORES):
        in_maps.append(
            {
                "x_in": np.ascontiguousarray(
                    xT[:, i * ROWS_PER_CORE : (i + 1) * ROWS_PER_CORE]
                ),
                "wl_in": WLp,
                "b_in": Bp,
                "fw_in": FWp,
            }
        )

    res = bass_utils.run_bass_kernel_spmd(
        nc,
        in_maps,
        core_ids=list(range(NCORES)),
        trace=_CACHE.get("trace", False),
        **_CACHE.get("run_kwargs", {}),
    )
    _CACHE["last_results"] = res

    # out_d is [65, 8192] per core: rows 0..63 = s*(out-c), row 64 = s
    out = np.concatenate(
        [np.asarray(res.results[i]["out"]).astype(np.float32).T for i in range(NCORES)],
        0,
    )
    out = out[:, 0:H] / out[:, H : H + 1]
    out = out + c[None, :]
    return out.astype(np.float32)
